# revision 47
# baseline (speedup 1.0000x reference)
"""Multi-head attention (B=4, S=2048, D=1024, H=16) on 8 TRN2 NeuronCores.

Sharding: 2D grid (batch x head-group). Core c = g*4 + b handles batch b and
head group g (8 heads = 512 of the 1024 embedding columns).

v2 kernel: all matmul operands bf16 (host pre-casts x^T and weights, so no
on-chip rounding passes); fp32 PSUM accumulation everywhere.

Per-core phases:
  1. x^T [1024, 2048] bf16 DMA'd per k-tile; V = x @ Wv computed directly in
     normal [k, d] layout (no PE transposes), evicted with bias into
     vones[128, kt, 8*65] bf16 where each head slot is [V_h | 1]. Then
     K^T / Q^T pair-0 tiles [128, 2048] bf16 (head 2p at rows 0-63, 2p+1 at
     64-127); Q scaled by 1/8 at eviction so exp needs no scale.
  2. Attention per pair, per q-chunk of 512: score slices (kt, head) stream
     through PSUM tiles of 2 resp. 3 slices ([128, {2,3}, 512] f32); the two
     heads' score matmuls (contraction 64, partition bases 0/64) run
     concurrently in the PE via row tiling. One EXP per tile (N=1024/1536)
     evicts to bf16 es. PV matmuls ([V_h|1] stationary, es moving) run one
     tile behind the scores (software pipelining, so the PE never waits on
     exp) and accumulate av[65, 512] per head over all 16 kt (row 64 =
     sumexp). Normalize via DMA-broadcast of sumexp + DVE reciprocal*mul
     into attnT bf16. Next pair's Q^T/K^T projection matmuls (and, during
     the last pair, output-projection chunks) are dribbled between tiles
     into the PE's exp-wait slack using the 1 spare PSUM bank.
  3. Remaining output projection out = sum_p attnT[p]^T @ Wo[p] at the end.
Host sums the two head-group partials per batch and adds bo.
"""
import numpy as np

B, S, D, H, DH = 4, 2048, 1024, 16, 64
NCORES = 8
GCOLS = D // 2          # 512 cols per head-group core
NPAIRS = GCOLS // 128   # 4 head-pairs per core
NKT = S // 128          # 16 k-tiles
DC = D // 8 // 16       # 8 contraction chunks of 128 for projections
DC = 8
NQQ = 4                 # q processed in 512-wide chunks
GROUPS = [2, 3] * 6 + [2]   # 32 (kt, head) slices per (pair, qq)

_COMPILED = None


def _build():
    import concourse.bass as bass
    import concourse.bacc as bacc
    import concourse.tile as tile
    from concourse import mybir
    from contextlib import ExitStack

    F32 = mybir.dt.float32
    BF16 = mybir.dt.bfloat16
    EXP = mybir.ActivationFunctionType.Exp
    ADD = mybir.AluOpType.add
    MULT = mybir.AluOpType.mult

    nc = bacc.Bacc("TRN2", target_bir_lowering=False, debug=False)
    # host pre-arranges everything partition-major so each partition's data
    # is one long contiguous DMA descriptor
    xT = nc.dram_tensor("xT", [128, 4, DC, S // 4], BF16,
                        kind="ExternalInput").ap()
    wq = nc.dram_tensor("wq", [128, DC, GCOLS], BF16,
                        kind="ExternalInput").ap()
    wk = nc.dram_tensor("wk", [128, DC, GCOLS], BF16,
                        kind="ExternalInput").ap()
    wv = nc.dram_tensor("wv", [128, DC, GCOLS], BF16,
                        kind="ExternalInput").ap()
    wo = nc.dram_tensor("wo", [128, NPAIRS, D], BF16,
                        kind="ExternalInput").ap()
    bq = nc.dram_tensor("bq", [GCOLS], F32, kind="ExternalInput").ap()
    bk = nc.dram_tensor("bk", [GCOLS], F32, kind="ExternalInput").ap()
    bv = nc.dram_tensor("bv", [GCOLS], F32, kind="ExternalInput").ap()
    out = nc.dram_tensor("out", [S, D], BF16, kind="ExternalOutput").ap()

    with tile.TileContext(nc) as tc, ExitStack() as outer:
        const = outer.enter_context(tc.tile_pool(name="const", bufs=1))
        persist = outer.enter_context(tc.tile_pool(name="persist", bufs=1))

        # --- resident inputs (bf16, DMA'd directly, no staging) ---
        # [128, quarter, dc, 512]: one 8KB descriptor per (partition, qtr)
        xT_sb = persist.tile([128, 4, DC, S // 4], BF16)
        for h in range(4):
            nc.sync.dma_start(out=xT_sb[:, h], in_=xT[:, h])

        def xt(dc, t0, width):
            """x^T chunk [128, width] for tokens [t0, t0+width)."""
            h, o = t0 // (S // 4), t0 % (S // 4)
            return xT_sb[:, h, dc, o:o + width]

        wq_sb = persist.tile([128, DC, GCOLS], BF16)
        wk_sb = persist.tile([128, DC, GCOLS], BF16)
        wv_sb = persist.tile([128, DC, GCOLS], BF16)
        wo_sb = persist.tile([128, NPAIRS, D], BF16)
        nc.scalar.dma_start(out=wv_sb, in_=wv)
        nc.scalar.dma_start(out=wk_sb, in_=wk)
        nc.scalar.dma_start(out=wq_sb, in_=wq)
        nc.scalar.dma_start(out=wo_sb, in_=wo)

        bq_sb = const.tile([128, NPAIRS], F32)
        bk_sb = const.tile([128, NPAIRS], F32)
        nc.scalar.dma_start(out=bq_sb, in_=bq.rearrange("(p r) -> r p", r=128))
        nc.scalar.dma_start(out=bk_sb, in_=bk.rearrange("(p r) -> r p", r=128))
        # bv broadcast-replicated across partitions: [128, 512]
        bv_bc = const.tile([128, GCOLS], F32)
        bv_rep = bass.AP(tensor=bv.tensor, offset=bv.offset,
                         ap=[[0, 128], [1, GCOLS]])
        nc.scalar.dma_start(out=bv_bc, in_=bv_rep)

        # V in normal layout with ones col per head: [128, kt, 8*65]
        vones = persist.tile([128, NKT, 8 * 65], BF16)
        v4 = vones.rearrange("p k (h c) -> p k h c", c=65)
        for kt in range(NKT):
            nc.vector.memset(v4[:, kt, :, 64:65], 1.0)
        # [1, 64] ones: stationary operand of the K=1 sumexp-broadcast matmul
        ones_row = const.tile([1, 64], BF16)
        nc.vector.memset(ones_row, 1.0)

        qt_sb = [persist.tile([128, S], BF16, name=f"qt{p}", tag=f"qt{p}")
                 for p in range(NPAIRS)]
        kt_sb = [persist.tile([128, S], BF16, name=f"kt{p}", tag=f"kt{p}")
                 for p in range(NPAIRS)]
        attnT = [persist.tile([128, S], BF16, name=f"attnT{p}",
                              tag=f"attnT{p}") for p in range(NPAIRS)]

        def emit_qk_chunk(pool, p, nm, w_sb, b_sb, dst, nt):
            """One 512-token chunk of a Q^T/K^T pair projection (eager)."""
            csl = slice(p * 128, (p + 1) * 128)
            nsl = slice(nt * 512, (nt + 1) * 512)
            ps = pool.tile([128, 512], F32, name="proj_ps", tag="proj")
            for dc in range(DC):
                nc.tensor.matmul(ps, w_sb[:, dc, csl], xt(dc, nt * 512, 512),
                                 start=(dc == 0), stop=(dc == DC - 1))
            if nm == "q":  # fold the 1/sqrt(dh)=1/8 softmax scale into Q
                nc.vector.tensor_scalar(out=dst[:, nsl], in0=ps,
                                        scalar1=b_sb[:, p:p + 1],
                                        scalar2=0.125, op0=ADD, op1=MULT)
            else:
                nc.vector.tensor_scalar(out=dst[:, nsl], in0=ps,
                                        scalar1=b_sb[:, p:p + 1],
                                        scalar2=None, op0=ADD)

        # ---------------- phase 1: V (all heads) + pair-0 K^T/Q^T ----------
        with ExitStack() as ph1:
            pwide = ph1.enter_context(
                tc.tile_pool(name="pwide", bufs=2, space="PSUM"))
            projps1 = ph1.enter_context(
                tc.tile_pool(name="projps1", bufs=2, space="PSUM"))

            # V = x @ Wv per 2-kt group: psum [128, 2, 512]
            bv_h = bv_bc.rearrange("p (h c) -> p h c", c=64)
            for kg in range(NKT // 2):
                pv = pwide.tile([128, 2, GCOLS], F32, name="pv", tag="pw")
                for j in range(2):
                    kt = kg * 2 + j
                    for dc in range(DC):
                        nc.tensor.matmul(
                            pv[:, j, :], xt(dc, kt * 128, 128),
                            wv_sb[:, dc, :],
                            start=(dc == 0), stop=(dc == DC - 1),
                            skip_group_check=True)
                pv_h = pv.rearrange("p j (h c) -> p j h c", c=64)
                for j in range(2):
                    kt = kg * 2 + j
                    nc.vector.tensor_tensor(
                        out=v4[:, kt, :, 0:64], in0=pv_h[:, j, :, :],
                        in1=bv_h, op=ADD)

            for nt in range(4):
                emit_qk_chunk(projps1, 0, "k", wk_sb, bk_sb, kt_sb[0], nt)
            for nt in range(4):
                emit_qk_chunk(projps1, 0, "q", wq_sb, bq_sb, qt_sb[0], nt)

        # ---------------- phase 2: attention (+ dribbled projections) ------
        with ExitStack() as ph2:
            scpool = ph2.enter_context(
                tc.tile_pool(name="scpool", bufs=1, space="PSUM"))
            avpool = ph2.enter_context(
                tc.tile_pool(name="avpool", bufs=1, space="PSUM"))
            projps2 = ph2.enter_context(
                tc.tile_pool(name="projps2", bufs=1, space="PSUM"))
            espool = ph2.enter_context(tc.tile_pool(name="espool", bufs=2))
            small = ph2.enter_context(tc.tile_pool(name="small", bufs=2))
            osb = ph2.enter_context(tc.tile_pool(name="osb", bufs=2))

            # --- dribble queue: closures emitting one instruction each ---
            dq = []

            def queue_qk_chunk(p, nm, w_sb, b_sb, dst, nt):
                csl = slice(p * 128, (p + 1) * 128)
                nsl = slice(nt * 512, (nt + 1) * 512)
                cell = {}

                def mk_mm(dc):
                    def f():
                        if dc == 0:
                            cell["ps"] = projps2.tile([128, 512], F32,
                                                      name="proj_ps",
                                                      tag="proj")
                        nc.tensor.matmul(cell["ps"], w_sb[:, dc, csl],
                                         xt(dc, nt * 512, 512),
                                         start=(dc == 0), stop=(dc == DC - 1),
                                         skip_group_check=True)
                    return f

                def mk_evict():
                    def f():
                        if nm == "q":
                            nc.vector.tensor_scalar(
                                out=dst[:, nsl], in0=cell["ps"],
                                scalar1=b_sb[:, p:p + 1],
                                scalar2=0.125, op0=ADD, op1=MULT)
                        else:
                            nc.vector.tensor_scalar(
                                out=dst[:, nsl], in0=cell["ps"],
                                scalar1=b_sb[:, p:p + 1],
                                scalar2=None, op0=ADD)
                    return f

                for dc in range(DC):
                    dq.append(mk_mm(dc))
                dq.append(mk_evict())

            def queue_out_chunk(qc):
                """Output projection for one 128-row q chunk."""
                cell = {}

                def mk_mm(p, nt):
                    def f():
                        if p == 0:
                            cell[nt] = projps2.tile([128, 512], F32,
                                                    name="o_ps", tag="proj")
                        nc.tensor.matmul(
                            cell[nt],
                            attnT[p][:, qc * 128:(qc + 1) * 128],
                            wo_sb[:, p, nt * 512:(nt + 1) * 512],
                            start=(p == 0), stop=(p == NPAIRS - 1),
                            skip_group_check=True)
                    return f

                def mk_evict(nt):
                    def f():
                        if nt == 0:
                            cell["o"] = osb.tile([128, 1024], BF16,
                                                 name="o_sb", tag="o_sb")
                        nc.vector.tensor_copy(
                            cell["o"][:, nt * 512:(nt + 1) * 512], cell[nt])
                        if nt == 1:  # one 4KB-per-row DMA for the full chunk
                            nc.sync.dma_start(
                                out=out[qc * 128:(qc + 1) * 128, :],
                                in_=cell["o"])
                    return f

                for nt in range(2):
                    for p in range(NPAIRS):
                        dq.append(mk_mm(p, nt))
                    dq.append(mk_evict(nt))

            def emit_pv(prev, av):
                pes, psl, pp = prev
                for j, (kt, hd) in enumerate(psl):
                    slot = 2 * pp + hd
                    nc.tensor.matmul(
                        av[hd], vones[:, kt, slot * 65:slot * 65 + 65],
                        pes[:, j, :],
                        start=(kt == 0), stop=(kt == NKT - 1),
                        skip_group_check=True)

            nq = []  # (min_tile_idx, closure): popped when ti >= min_ti

            def queue_norm(av, pp, qsl, qqi):
                # No DMA anywhere on this path: evict both av banks, take
                # reciprocals of the [1, 512] sumexp rows (pure DVE), then
                # broadcast via a K=1 PE matmul through the proj bank and
                # multiply. The broadcast+mul units go into dq, whose pops
                # are chunk-FIFO-ordered, so the K=1 matmul (start=True
                # clears its whole PSUM bank) can never interleave another
                # chunk's in-flight accumulation in that bank.
                av_sbs, rrs = [], []

                def c1():
                    for hd in range(2):
                        av_sb = small.tile([65, 512], F32, name="av_sb",
                                           tag=f"av_sb{hd}")
                        nc.vector.tensor_copy(av_sb, av[hd])
                        av_sbs.append(av_sb)

                def c2():
                    for hd in range(2):
                        # bf16 copy of the raw sumexp row (bcast operand)
                        rr = small.tile([1, 512], BF16, name="rr",
                                        tag=f"rr{hd}")
                        nc.vector.tensor_copy(rr, av_sbs[hd][64:65, :])
                        rrs.append(rr)
                    dq.append(mk_fin(0))
                    dq.append(mk_fin(1))
                    if pp == NPAIRS - 1 and 1 <= qqi <= 2:
                        # last pair: dribble out-proj chunks one qq behind
                        # the attnT writes; the rest runs in phase 3 where
                        # 4 PSUM banks make it cheaper than 1-bank dribbling
                        for qc in range((qqi - 1) * 4, qqi * 4):
                            queue_out_chunk(qc)

                def mk_fin(hd):
                    def f():
                        # broadcast sumexp to 64 partitions via K=1 matmul,
                        # then reciprocal on the (HW-safe) [64, 512] shape
                        bc_ps = projps2.tile([64, 512], F32, name="bc_ps",
                                             tag="proj")
                        nc.tensor.matmul(bc_ps, ones_row, rrs[hd],
                                         start=True, stop=True)
                        rec = small.tile([64, 512], F32, name="rec",
                                         tag=f"rec{hd}")
                        nc.vector.reciprocal_approx_fast(out=rec, in_=bc_ps)
                        if hd == 0:
                            nc.vector.tensor_mul(attnT[pp][0:64, qsl],
                                                 av_sbs[0][0:64, :], rec)
                        else:
                            tmp = small.tile([64, 512], BF16, name="tmp",
                                             tag="tmp")
                            nc.vector.tensor_mul(tmp, av_sbs[1][0:64, :],
                                                 rec)
                            nc.sync.dma_start(out=attnT[pp][64:128, qsl],
                                              in_=tmp)
                    return f

                nq.extend([(0, c1), (1, c2)])

            prev = None   # (es, sl, p): PV runs one tile behind scores
            pending = None  # (av, p, qsl) from the previous qq

            for p in range(NPAIRS):
                for qq in range(NQQ):
                    qsl = slice(qq * 512, (qq + 1) * 512)
                    # refill dribble queue at qq boundaries
                    if p + 1 < NPAIRS:
                        queue_qk_chunk(p + 1, "k", wk_sb, bk_sb,
                                       kt_sb[p + 1], qq)
                        queue_qk_chunk(p + 1, "q", wq_sb, bq_sb,
                                       qt_sb[p + 1], qq)

                    av = [avpool.tile([65, 512], F32, name=f"av{h}",
                                      tag=f"av{h}") for h in range(2)]
                    # Score matmuls are emitted in strict (head0, head1)
                    # pairs -- even across scq-tile boundaries -- so every
                    # adjacent PE pair row-packs. A tile's exp/PV/
                    # housekeeping flushes only after the pair completes.
                    tile_of, base_of = [], {}
                    s0 = 0
                    for t, gsz in enumerate(GROUPS):
                        base_of[t] = s0
                        tile_of += [t] * gsz
                        s0 += gsz
                    tiles_open = {}
                    flushed = [0]

                    def sc_emit(s):
                        t = tile_of[s]
                        if t not in tiles_open:
                            gsz = GROUPS[t]
                            tiles_open[t] = {
                                "scq": scpool.tile([128, gsz, 512], F32,
                                                   name="scq",
                                                   tag=f"scq{gsz}"),
                                "es": espool.tile([128, gsz, 512], BF16,
                                                  name="es",
                                                  tag=f"es{gsz}"),
                                "sl": []}
                        cur = tiles_open[t]
                        kt, hd = s >> 1, s & 1
                        rows = slice(hd * 64, hd * 64 + 64)
                        nc.tensor.matmul(
                            cur["scq"][:, s - base_of[t], :],
                            kt_sb[p][rows, kt * 128:(kt + 1) * 128],
                            qt_sb[p][rows, qsl],
                            start=True, stop=True)
                        cur["sl"].append((kt, hd))

                    def flush_tiles():
                        nonlocal prev, pending
                        t = flushed[0]
                        while (t < len(GROUPS) and t in tiles_open
                               and len(tiles_open[t]["sl"]) == GROUPS[t]):
                            cur = tiles_open.pop(t)
                            if prev is not None:
                                emit_pv(prev, pending[0]
                                        if t == 0 and pending else av)
                                if t == 0 and pending:
                                    queue_norm(*pending)
                                    pending = None
                            nc.scalar.activation(cur["es"], cur["scq"], EXP)
                            while nq and nq[0][0] <= t:
                                nq.pop(0)[1]()
                            for _ in range(2 if p + 1 < NPAIRS else 4):
                                if dq:
                                    dq.pop(0)()
                            prev = (cur["es"], cur["sl"], p)
                            t += 1
                        flushed[0] = t

                    for i in range(16):
                        sc_emit(2 * i)
                        sc_emit(2 * i + 1)
                        flush_tiles()
                    pending = (av, p, qsl, qq)

            # drain the pipeline tail and leftover dribble ops
            emit_pv(prev, pending[0])
            queue_norm(*pending)
            while nq:
                nq.pop(0)[1]()
            while dq:
                dq.pop(0)()

        # ------------- phase 3: remaining output projection (wide psum) ----
        with ExitStack() as fin:
            osb2 = fin.enter_context(tc.tile_pool(name="osb2", bufs=4))
            psout = fin.enter_context(
                tc.tile_pool(name="psout", bufs=4, space="PSUM"))
            for qc in range(8, S // 128):
                o_ps = [psout.tile([128, 512], F32, name=f"o_ps{nt}",
                                   tag="psout") for nt in range(2)]
                for p in range(NPAIRS):
                    for nt in range(2):
                        nc.tensor.matmul(
                            o_ps[nt],
                            attnT[p][:, qc * 128:(qc + 1) * 128],
                            wo_sb[:, p, nt * 512:(nt + 1) * 512],
                            start=(p == 0), stop=(p == NPAIRS - 1),
                            skip_group_check=True)
                o_sb = osb2.tile([128, 1024], BF16, name="o_sb", tag="o_sb")
                for nt in range(2):
                    nc.vector.tensor_copy(o_sb[:, nt * 512:(nt + 1) * 512],
                                          o_ps[nt])
                nc.sync.dma_start(out=out[qc * 128:(qc + 1) * 128, :],
                                  in_=o_sb)

    nc.compile()
    return nc


def _get_compiled():
    global _COMPILED
    if _COMPILED is None:
        _COMPILED = _build()
    return _COMPILED


def make_in_maps(**inputs):
    import ml_dtypes
    bf16 = ml_dtypes.bfloat16
    x = np.asarray(inputs["inputs"], np.float32)
    # xT partition-major: [p, qtr, dc, n] so each (p, qtr) is contiguous
    xTb = [np.ascontiguousarray(
               x[b].T.reshape(DC, 128, 4, S // 4).transpose(1, 2, 0, 3)
           ).astype(bf16) for b in range(B)]
    gslice = {}
    for nm in ("Wq", "Wk", "Wv", "Wo", "bq", "bk", "bv"):
        a = np.asarray(inputs[nm], np.float32)
        for g in range(2):
            sl = slice(g * GCOLS, (g + 1) * GCOLS)
            if nm == "Wo":
                # [r, pair, n]: partition-major, 8KB contiguous per row
                gslice[(nm, g)] = np.ascontiguousarray(
                    a[sl, :].reshape(NPAIRS, 128, D).transpose(1, 0, 2)
                ).astype(bf16)
            elif nm.startswith("W"):
                # [p, dc, m]: partition-major
                gslice[(nm, g)] = np.ascontiguousarray(
                    a[:, sl].reshape(DC, 128, GCOLS).transpose(1, 0, 2)
                ).astype(bf16)
            else:
                gslice[(nm, g)] = np.ascontiguousarray(a[sl])
    in_maps = []
    for c in range(NCORES):
        g, b = c // B, c % B
        in_maps.append({
            "xT": xTb[b],
            "wq": gslice[("Wq", g)], "wk": gslice[("Wk", g)],
            "wv": gslice[("Wv", g)], "wo": gslice[("Wo", g)],
            "bq": gslice[("bq", g)], "bk": gslice[("bk", g)],
            "bv": gslice[("bv", g)],
        })
    return in_maps


def combine(results, bo):
    out = np.empty((B, S, D), np.float32)
    bo = np.asarray(bo, np.float32)
    for b in range(B):
        out[b] = (results[b]["out"].astype(np.float32)
                  + results[B + b]["out"].astype(np.float32) + bo)
    return out


def kernel(**inputs):
    from concourse import bass_utils
    nc = _get_compiled()
    in_maps = make_in_maps(**inputs)
    res = bass_utils.run_bass_kernel_spmd(
        nc, in_maps, core_ids=list(range(NCORES)))
    return combine(res.results, inputs["bo"])
